# revision 1
# baseline (speedup 1.0000x reference)
"""Bass/Trainium2 kernel for masked (padding) multi-head self-attention.

Problem: B=2, T=2048, C=1024, H=16 heads of DH=64.
  q/k/v = x @ W* + b*  ->  att = softmax(mask(q k^T / 8))  ->  y = att @ v

Sharding over 8 NeuronCores: core = (batch b, head-group hg) with
b = core // 4, hg = core % 4; each core computes 4 heads for one batch
element (its [T, 256] slice of q/k/v from the Wq/Wk/Wv column slice).

Host-side preprocessing (inside kernel()):
  - Tokens with mask==0 contribute nothing (their att rows are zeroed by
    the reference, and their columns get -inf scores), so we gather only
    the valid tokens per batch and pad to a multiple of 128.  This
    roughly halves T and quarters the T x T attention work.
  - x is gathered+transposed on the host to x^T [C, T_pad], so the
    device needs no transposes at all:
      qT[d,t] = sum_c Wq[c,d] xT[c,t]     (lhsT=Wq tile,  rhs=xT)
      v[t,d]  = sum_c xT[c,t] Wv[c,d]     (lhsT=xT tile,  rhs=Wv)
      sT[k,q] = sum_d kT[d,k] qT[d,q]     (lhsT=kT slice, rhs=qT)   [k on partitions]
      e       = exp(0.125*sT + bias_k)    (bias_k = -1e30 on pad rows)
      yT[d,q] = sum_k vaug[k,d] e[k,q]    (lhsT=[v | 1] (M=65), rhs=e)
    Row 64 of the AV output is sum_k e[k,q] = the softmax denominator.
  - Normalization (divide by denominator), the final transpose back to
    [T, C] layout, and zeroing of masked query rows happen on the host
    during unsharding.

All matmuls run as float32r (full-rate fp32, ~tf32-ish rounding).

Performance profile (instruction-cost-model timeline, seed-0 mask, 1152
padded tokens/core; HW-validated for correctness at rel err 3.8e-4):
  ~77.5 us/core total.
  head  ~19 us: 4.7MB x^T DMA at the 360GB/s roofline gates the first
                exp (every matmul contracts over all of C); qk d-tile-0
                projection + PE warmup dummies pipeline underneath it.
  middle ~52us: jointly PE/ACT bound. ACT: 36 exp instrs (~41 us busy,
                1 elem/lane/cycle floor) run dense through the
                head-0/1 score phase; PE: ~61 us busy at the fp32r
                cycle floor (projections K=128, scores K=64 emitted
                row-group-paired for silicon concurrency, AV M=65 with
                the softmax denominator as a free stationary column).
                ~8 us of ACT holes in the tail groups reflect PE
                oversubscription (34.5 vs 22.6 us in the g-loop) and
                would need a 9th PSUM bank to close.
  tail  ~7 us:  exp(last k-tile) -> final AV group -> accumulator adds
                -> per-chunk output DMAs (HWDGE-queue paced) -> drain.
Known-negative experiments (reverted): DVE-side denominator to enable
AV column-packing (+27 us: DVE re-touches every exp element); splitting
the final AV group (+3 us: serializes accumulator read-modify-writes);
emission-order rotations (bit-identical schedules - Tile's scheduler
already hoists everything PSUM slots allow).
"""

import math
import sys

sys.path.insert(0, "/opt/trn_rl_repo")

import numpy as np

import concourse.bacc as bacc
import concourse.mybir as mybir
import concourse.tile as tile
from concourse import bass_utils

F32 = mybir.dt.float32
F32R = mybir.dt.float32r
AF = mybir.ActivationFunctionType

B, T, C, H = 2, 2048, 1024, 16
DH = C // H            # 64
HPC = 4                # heads per core
CSL = HPC * DH         # 256, per-core column slice of C
N_CORES = 8
GROUP = 3              # k-tiles per AV psum accumulation group

_CACHE: dict = {}


# valid k-tile counts: q-chunks of 256..512 in <=3 chunks (PSUM bank limit)
_VALID_NKT = {2: 1, 3: 1, 4: 1, 6: 2, 8: 2, 9: 3}


def _pick_dims(max_valid: int):
    """T_pad (multiple of 128, sized so uniform q-chunks of 256..512 exist
    and the score PSUM tile stays within 3 banks)."""
    nkt = max(2, math.ceil(max_valid / 128))
    while nkt not in _VALID_NKT:
        nkt += 1
        if nkt > 9:
            raise NotImplementedError(
                f"too many valid tokens ({max_valid}) for on-chip layout")
    nch = _VALID_NKT[nkt]
    tp = nkt * 128
    cw = tp // nch
    return tp, nkt, cw, nch


def _build(tp: int, nkt: int, cw: int, nch: int, with_bv: bool = True):
    nc = bacc.Bacc("TRN2", target_bir_lowering=False, debug=False,
                   num_devices=N_CORES)

    xt_d = nc.dram_tensor("xt", [C, tp], F32, kind="ExternalInput")
    wq_d = nc.dram_tensor("wq", [C, CSL], F32, kind="ExternalInput")
    wk_d = nc.dram_tensor("wk", [C, CSL], F32, kind="ExternalInput")
    wv_d = nc.dram_tensor("wv", [C, CSL], F32, kind="ExternalInput")
    # misc128: col 0..3 = bqk (bq d0, bq d1, bk d0, bk d1), col 4.. = ebias
    misc128_d = nc.dram_tensor("misc128", [128, 4 + nkt], F32, kind="ExternalInput")
    # misc1: [0:CSL] = bv, [CSL:CSL+128] = ones row
    misc1_d = nc.dram_tensor("misc1", [1, CSL + 128], F32, kind="ExternalInput")
    onesv_d = nc.dram_tensor("onesv", [128, nkt * HPC], F32, kind="ExternalInput")
    out_d = nc.dram_tensor("out", [DH + 1, HPC, tp], F32, kind="ExternalOutput")

    NCT = C // 128  # 8 contraction tiles over C
    chunks = [(j * cw, cw) for j in range(nch)]

    with tile.TileContext(nc) as tc:
        with tc.tile_pool(name="const", bufs=1) as cp:
            # ---- resident tiles
            xt_sb = cp.tile([128, NCT, tp], F32R, tag="xt")
            wq_sb = cp.tile([128, NCT, CSL], F32R, tag="wq")
            wk_sb = cp.tile([128, NCT, CSL], F32R, tag="wk")
            wv_sb = cp.tile([128, NCT, CSL], F32R, tag="wv")
            misc128_sb = cp.tile([128, 4 + nkt], F32, tag="misc128")
            misc1_sb = cp.tile([1, CSL + 128], F32R, tag="misc1")
            qt_sb = [cp.tile([128, tp], F32R, tag=f"qt{p}", name=f"qt{p}") for p in range(2)]
            kt_sb = [cp.tile([128, tp], F32R, tag=f"kt{p}", name=f"kt{p}") for p in range(2)]
            v_sb = cp.tile([128, nkt, HPC, DH + 1], F32R, tag="v")
            acc_sb = cp.tile([DH + 1, HPC, tp], F32, tag="acc")
            bqk_sb = misc128_sb[:, 0:4]
            ebias_sb = misc128_sb[:, 4:4 + nkt]
            bv_sb = misc1_sb[:, 0:CSL]
            ones_sb = misc1_sb[:, CSL:CSL + 128]

            scratch = cp.tile([1, 8], F32, tag="scratch")

            xt_r = xt_d.ap().rearrange("(i p) t -> p i t", p=128).bitcast(F32R)
            wq_r = wq_d.ap().rearrange("(i p) d -> p i d", p=128).bitcast(F32R)
            wk_r = wk_d.ap().rearrange("(i p) d -> p i d", p=128).bitcast(F32R)
            wv_r = wv_d.ap().rearrange("(i p) d -> p i d", p=128).bitcast(F32R)
            # critical-path DMAs first: d-tile-0 halves of Wq/Wk, then xt
            # per c-tile so qT/kT accumulation pipelines with the transfers.
            nc.sync.dma_start(wq_sb[:, :, 0:128], wq_r[:, :, 0:128])
            for i in range(NCT // 2):
                nc.sync.dma_start(xt_sb[:, i, :], xt_r[:, i, :])
            nc.sync.dma_start(wk_sb[:, :, 0:128], wk_r[:, :, 0:128])
            for i in range(NCT // 2, NCT):
                nc.sync.dma_start(xt_sb[:, i, :], xt_r[:, i, :])
            nc.sync.dma_start(misc128_sb[:], misc128_d.ap()[:])
            nc.sync.dma_start(wq_sb[:, :, 128:256], wq_r[:, :, 128:256])
            nc.sync.dma_start(wk_sb[:, :, 128:256], wk_r[:, :, 128:256])
            nc.sync.dma_start(wv_sb[:], wv_r[:])
            nc.sync.dma_start(misc1_sb[:], misc1_d.ap()[:].bitcast(F32R))
            nc.sync.dma_start(
                v_sb[:, :, :, DH],
                onesv_d.ap().rearrange("p (t h) -> p t h", h=HPC).bitcast(F32R))

            # warm the ACT exp table during the DMA window
            nc.gpsimd.memset(scratch[:], 0.0)
            nc.scalar.activation(scratch[:], scratch[:], AF.Exp)

            n_groups = math.ceil(nkt / GROUP)
            seq_heads = nkt >= 12  # SBUF can't hold two heads of e-tiles
            ebufs = (GROUP + 3) if seq_heads else (2 * nkt + 1)

            def make_proj_qk(pool, tag, split_evac=False):
                def proj_qk(p):
                    n = 0
                    for w_sb, o_sb, bcol in ((wq_sb, qt_sb, 0), (wk_sb, kt_sb, 2)):
                        for off, w in chunks:
                            ps = pool.tile([128, cw], F32, tag=tag, name="pqk")
                            for ct in range(NCT):
                                nc.tensor.matmul(
                                    ps[:, 0:w],
                                    w_sb[:, ct, p * 128:(p + 1) * 128],
                                    xt_sb[:, ct, off:off + w],
                                    start=(ct == 0), stop=(ct == NCT - 1),
                                )
                            bias_ap = bqk_sb[:, bcol + p:bcol + p + 1]
                            if split_evac and n % 2 == 0:
                                nc.scalar.activation(
                                    o_sb[p][:, off:off + w], ps[:, 0:w],
                                    AF.Identity, bias=bias_ap)
                            else:
                                nc.vector.tensor_scalar_add(
                                    o_sb[p][:, off:off + w], ps[:, 0:w],
                                    bias_ap)
                            n += 1
                return proj_qk

            # phase A: qkT d-tile-0 projection with 6 psum slots so all six
            # accumulation groups pipeline with the incoming xt DMAs.
            with tc.tile_pool(name="pa", bufs=6, space="PSUM") as pa:
                # warm the PE (HAM clock gate) during the DMA window
                wsc = cp.tile([128, 16], F32, tag="wsc")
                nc.gpsimd.memset(wsc[:], 0.0)
                for _ in range(45):
                    wps = pa.tile([16, 16], F32, tag="a", name="wps")
                    nc.tensor.matmul(wps[:], wsc[:, 0:16], wsc[:],
                                     start=True, stop=True)
                make_proj_qk(pa, "a", split_evac=True)(0)

            with (
                tc.tile_pool(name="ops", bufs=2, space="PSUM") as ops,
                tc.tile_pool(name="sps", bufs=2, space="PSUM") as sps_pool,
                tc.tile_pool(name="epool", bufs=ebufs) as ep,
            ):
                e_tiles: dict = {}
                proj_qk = make_proj_qk(ops, "o")

                def proj_v(tts):
                    for t in tts:
                        ps = ops.tile([128, CSL], F32, tag="o", name="pv")
                        for ct in range(NCT):
                            nc.tensor.matmul(
                                ps[:],
                                xt_sb[:, ct, t * 128:(t + 1) * 128],
                                wv_sb[:, ct, :],
                                start=(ct == 0),
                                stop=(not with_bv and ct == NCT - 1),
                            )
                        if with_bv:
                            nc.tensor.matmul(ps[:], ones_sb[:], bv_sb[:],
                                             start=False, stop=True)
                        nc.vector.tensor_copy(
                            v_sb[:, t, :, 0:DH],
                            ps[:].rearrange("p (h d) -> p h d", h=HPC),
                        )

                def scores(h, tts, filler=None):
                    pd, po = h // 2, (h % 2) * 64
                    qt_h, kt_h = qt_sb[pd], kt_sb[pd]
                    for t in tts:
                        if filler:
                            filler(t)
                        ps = sps_pool.tile([128, nch, 512], F32, tag="s",
                                           name="sps")
                        for j, (off, w) in enumerate(chunks):
                            nc.tensor.matmul(
                                ps[:, j, 0:w],
                                kt_h[po:po + 64, t * 128:(t + 1) * 128],
                                qt_h[po:po + 64, off:off + w],
                                start=True, stop=True,
                            )
                        e_t = ep.tile([128, nch, cw], F32R, tag="e", name="e")
                        nc.scalar.activation(
                            e_t[:], ps[:, :, 0:cw], AF.Exp,
                            bias=ebias_sb[:, t:t + 1], scale=0.125,
                        )
                        e_tiles[(h, t)] = e_t

                def scores_pair(hA, hB, t, filler=None):
                    # hA/hB share a qT/kT d-tile at partition offsets 0/64;
                    # alternating the chunk matmuls lets the PE row-groups
                    # overlap the two heads' streams.
                    if filler:
                        filler(t)
                    pd = hA // 2
                    qt_h, kt_h = qt_sb[pd], kt_sb[pd]
                    pss = {}
                    for h in (hA, hB):
                        pss[h] = sps_pool.tile([128, nch, 512], F32, tag="s",
                                               name="sps")
                    for j, (off, w) in enumerate(chunks):
                        for h in (hA, hB):
                            po = (h % 2) * 64
                            nc.tensor.matmul(
                                pss[h][:, j, 0:w],
                                kt_h[po:po + 64, t * 128:(t + 1) * 128],
                                qt_h[po:po + 64, off:off + w],
                                start=True, stop=True,
                            )
                    for h in (hA, hB):
                        e_t = ep.tile([128, nch, cw], F32R, tag="e", name="e")
                        nc.scalar.activation(
                            e_t[:], pss[h][:, :, 0:cw], AF.Exp,
                            bias=ebias_sb[:, t:t + 1], scale=0.125,
                        )
                        e_tiles[(h, t)] = e_t

                def av(h, g, only_j=None):
                        tts = range(g * GROUP, min((g + 1) * GROUP, nkt))
                        for j, (off, w) in enumerate(chunks):
                            if only_j is not None and j != only_j:
                                continue
                            avp = ops.tile([DH + 1, cw], F32, tag="o", name="av")
                            for i, t in enumerate(tts):
                                nc.tensor.matmul(
                                    avp[:],
                                    v_sb[:, t, h, :],
                                    e_tiles[(h, t)][:, j, :],
                                    start=(i == 0), stop=(t == tts[-1]),
                                )
                            if g == 0:
                                nc.vector.tensor_copy(
                                    acc_sb[:, h, off:off + w], avp[:])
                            else:
                                nc.vector.tensor_add(
                                    acc_sb[:, h, off:off + w],
                                    acc_sb[:, h, off:off + w], avp[:])
                            if g == n_groups - 1:
                                # chunk complete: stream it out now
                                nc.sync.dma_start(
                                    out_d.ap()[:, h, off:off + w],
                                    acc_sb[:, h, off:off + w])

                def grp(g):
                    return range(g * GROUP, min((g + 1) * GROUP, nkt))

                if seq_heads:
                    # simple sequential-head schedule (larger T_pad): exps
                    # pace the kernel; e-tile footprint stays ~GROUP tiles.
                    proj_qk(1)
                    for g in range(n_groups):
                        proj_v(grp(g))
                    for h in range(HPC):
                        for g in range(n_groups):
                            scores(h, grp(g))
                            av(h, g)
                else:
                    # drip-feed qkd1 accumulation groups between score tiles
                    # so the PE has work while score psum slots wait on exps
                    qkd1_units = []
                    for w_sb, o_sb, bcol in ((wq_sb, qt_sb, 0),
                                             (wk_sb, kt_sb, 2)):
                        for off, w in chunks:
                            qkd1_units.append((w_sb, o_sb, bcol, off, w))

                    def emit_qkd1_unit(t):
                        if not qkd1_units or (t is not None and t < 2):
                            return
                        w_sb, o_sb, bcol, off, w = qkd1_units.pop(0)
                        ps = ops.tile([128, cw], F32, tag="o", name="pqk1")
                        for ct in range(NCT):
                            nc.tensor.matmul(
                                ps[:, 0:w],
                                w_sb[:, ct, 128:256],
                                xt_sb[:, ct, off:off + w],
                                start=(ct == 0), stop=(ct == NCT - 1),
                            )
                        nc.vector.tensor_scalar_add(
                            o_sb[1][:, off:off + w], ps[:, 0:w],
                            bqk_sb[:, bcol + 1:bcol + 2])

                    for t in range(nkt):
                        scores_pair(0, 1, t, filler=emit_qkd1_unit)
                    while qkd1_units:
                        emit_qkd1_unit(None)
                    # interleave v-projection, AV and scores(2,3) per k-tile
                    # group: AV(0..1,g) frees head-0/1 e-tiles as head-2/3's
                    # are produced; av(2,g-1)/av(3,g-1) trail a group behind.
                    for g in range(n_groups):
                        proj_v(grp(g))
                        av(0, g)
                        av(1, g)
                        for t in grp(g):
                            scores_pair(2, 3, t)
                        # non-critical trailing AV demoted below the pairs:
                        # the scheduler still hoists it into slot-wait idle,
                        # but ACT-critical score pairs win priority ties
                        if g > 0:
                            av(2, g - 1)
                            av(3, g - 1)
                    av(2, n_groups - 1)
                    av(3, n_groups - 1)

    nc.compile()
    return nc


def _get_nc(tp, nkt, cw, nch, with_bv=True):
    key = (tp, nkt, cw, nch, with_bv)
    if key not in _CACHE:
        _CACHE[key] = _build(tp, nkt, cw, nch, with_bv)
    return _CACHE[key]


def kernel(x, Wq, bq, Wk, bk, Wv, bv, mask):
    x = np.asarray(x, dtype=np.float32)
    Wq = np.asarray(Wq, dtype=np.float32)
    bq = np.asarray(bq, dtype=np.float32)
    Wk = np.asarray(Wk, dtype=np.float32)
    bk = np.asarray(bk, dtype=np.float32)
    Wv = np.asarray(Wv, dtype=np.float32)
    bv = np.asarray(bv, dtype=np.float32)
    mask = np.asarray(mask)

    idxs = [np.nonzero(mask[b] != 0)[0] for b in range(B)]
    tvs = [len(ix) for ix in idxs]
    tp, nkt, cw, nch = _pick_dims(max(max(tvs), 1))
    with_bv = bool(np.any(bv))
    nc = _get_nc(tp, nkt, cw, nch, with_bv)

    onesv = np.ones((128, nkt * HPC), np.float32)

    # per-batch tensors
    xts, ebs = [], []
    for b in range(B):
        xt = np.zeros((C, tp), np.float32)
        if tvs[b]:
            xt[:, :tvs[b]] = x[b][idxs[b]].T
        xts.append(xt)
        eb = np.full(tp, -1e30, np.float32)
        eb[:tvs[b]] = 0.0
        ebs.append(eb.reshape(nkt, 128).T.copy())

    in_maps = []
    for core in range(N_CORES):
        b, hg = core // HPC, core % HPC
        cs = hg * CSL
        misc128 = np.concatenate([
            np.stack([bq[cs:cs + 128], bq[cs + 128:cs + 256],
                      bk[cs:cs + 128], bk[cs + 128:cs + 256]], axis=1),
            ebs[b],
        ], axis=1)
        misc1 = np.concatenate([bv[cs:cs + CSL],
                                np.ones(128, np.float32)]).reshape(1, -1)
        in_maps.append({
            "xt": xts[b],
            "wq": np.ascontiguousarray(Wq[:, cs:cs + CSL]),
            "wk": np.ascontiguousarray(Wk[:, cs:cs + CSL]),
            "wv": np.ascontiguousarray(Wv[:, cs:cs + CSL]),
            "misc128": np.ascontiguousarray(misc128),
            "misc1": np.ascontiguousarray(misc1),
            "onesv": onesv,
        })

    try:
        res = bass_utils.run_bass_kernel_spmd(
            nc, in_maps, core_ids=list(range(N_CORES)), trace=False)
    except Exception:
        # transient axon-worker/NRT failures recover on retry
        res = bass_utils.run_bass_kernel_spmd(
            nc, in_maps, core_ids=list(range(N_CORES)), trace=False)

    y = np.zeros((B, T, C), np.float32)
    for core in range(N_CORES):
        b, hg = core // HPC, core % HPC
        out = res.results[core]["out"]          # [DH+1, HPC, tp]
        ix, tv = idxs[b], tvs[b]
        if not tv:
            continue
        for h in range(HPC):
            numer = out[:DH, h, :tv]
            denom = out[DH, h, :tv]
            col = hg * CSL + h * DH
            y[b, ix, col:col + DH] = (numer / denom).T
    return y

